# revision 54
# baseline (speedup 1.0000x reference)
"""Causal multi-head attention (B=2, H=16, S=2048, F=128) on 8 TRN2 NeuronCores.

Sharding: tensor-parallel over the (batch, head) axis — 32 independent
(b, h) attention problems, 4 per core. No collectives needed.

Per-head on-chip algorithm (all layouts chosen so no on-chip transposes
are ever required):
  - host pre-transposes x to xT [F, S] per head, and W to WT [f, e].
  - QT = WqT.T @ xT   (PSUM) + bias -> SBUF   [e=128, s=2048]
  - KT likewise.
  - V' = xT_tile.T @ [WvT | 0] + [bv | 1]     [s=128, e'=129] per s-tile
    (extra ones column makes the AV matmul also produce softmax denominators)
  - For each k-tile (128 keys), compute ST = K_tile . Q  ->  [k=128, q<=1024]
    strips in PSUM, exp on ACT -> PT (bf16) in SBUF, lower-triangle zero-mask
    on the diagonal block, then AV: out_acc[q,129] += PT_tile.T @ V'_tile,
    accumulated in PSUM over k-tiles. Column 128 of the accumulator is
    sum(exp) — normalize with DVE reciprocal + tensor_scalar multiply.
  - Causality: only k-tiles with k <= q are ever computed.
"""

import math

import numpy as np
import ml_dtypes

import concourse.bass as bass
import concourse.tile as tile
import concourse.mybir as mybir
from concourse import bacc, bass_utils

B, H, S, F = 2, 16, 2048, 128
NCORES = 8
HPC = (B * H) // NCORES  # (b,h) pairs per core
SCALE = 1.0 / math.sqrt(F)
HALF = S // 2  # q processed in two 1024-wide halves (PSUM budget)
GSTRIDE = 136  # col stride of packed AV accumulator groups (32B aligned)

QK_FP32R = False  # scores/projection matmuls in fp32r (else bf16)

_cache = {}


def _build():
    f32 = mybir.dt.float32
    bf16 = mybir.dt.bfloat16
    f32r = mybir.dt.float32r
    qk_dt = f32r if QK_FP32R else bf16
    Exp = mybir.ActivationFunctionType.Exp

    nc = bacc.Bacc("TRN2")

    if QK_FP32R:
        xtq = nc.dram_tensor("xt32", [HPC, F, S], f32r, kind="ExternalInput")
    xtb = nc.dram_tensor("xtbh", [HPC, F, S], bf16, kind="ExternalInput")
    wqt = nc.dram_tensor("wqt", [HPC, F, F], qk_dt, kind="ExternalInput")
    wkt = nc.dram_tensor("wkt", [HPC, F, F], qk_dt, kind="ExternalInput")
    wvt = nc.dram_tensor("wvt", [HPC, F, F + 1], bf16, kind="ExternalInput")
    bqt = nc.dram_tensor("bqt", [F, HPC], f32, kind="ExternalInput")
    bkt = nc.dram_tensor("bkt", [F, HPC], f32, kind="ExternalInput")
    bvb = nc.dram_tensor("bvb", [HPC, GSTRIDE + F + 1], bf16,
                         kind="ExternalInput")
    msk = nc.dram_tensor("msk", [F, F], bf16, kind="ExternalInput")
    one = nc.dram_tensor("one", [1, F], bf16, kind="ExternalInput")
    out = nc.dram_tensor("out", [HPC, S, F], f32, kind="ExternalOutput")
    if not QK_FP32R:
        xtq = xtb

    with tile.TileContext(nc) as tc, \
            tc.tile_pool(name="consts", bufs=1) as consts, \
            tc.tile_pool(name="xin", bufs=2) as xin, \
            tc.tile_pool(name="qk", bufs=2) as qkp, \
            tc.tile_pool(name="vp", bufs=2 * (S // F)) as vpp, \
            tc.tile_pool(name="pt", bufs=7) as ptp, \
            tc.tile_pool(name="outs", bufs=6) as outp, \
            tc.tile_pool(name="st", bufs=2, space="PSUM") as stp, \
            tc.tile_pool(name="av", bufs=3, space="PSUM") as avp, \
            tc.tile_pool(name="vq", bufs=1, space="PSUM") as vqp:

        c_bq = consts.tile([F, HPC], f32, tag="bq")
        nc.gpsimd.dma_start(out=c_bq, in_=bqt[:, :])
        c_bk = consts.tile([F, HPC], f32, tag="bk")
        nc.gpsimd.dma_start(out=c_bk, in_=bkt[:, :])
        c_mask = consts.tile([F, F], bf16, tag="msk")
        nc.gpsimd.dma_start(out=c_mask, in_=msk[:, :])
        c_one = consts.tile([1, F], bf16, tag="one")
        nc.gpsimd.dma_start(out=c_one, in_=one[:, :])

        # deferred AV-batch emission, two strips deep: by the time an AV
        # batch is emitted, the exp it reads finished ~2 iterations ago,
        # so the PE never stalls waiting on ACT
        SKEW = 5
        pending = []

        def flush_pending(keep=0):
            while len(pending) > keep:
                pending.pop(0)()

        def make_prelude(hd):
            """Emission closures for head hd's input DMAs, QT/KT and V'
            projections. Popped one-per-ki during head hd-1's k-loop so
            this work hides under the previous head's softmax."""
            st8 = {"vtiles": []}

            def dmas(hd=hd):
                # halves so the first QK chunk can start after half a load
                wq = xin.tile([F, F], qk_dt, tag="wq", name=f"wq_{hd}")
                nc.sync.dma_start(out=wq, in_=wqt[hd])
                wk = xin.tile([F, F], qk_dt, tag="wk", name=f"wk_{hd}")
                nc.sync.dma_start(out=wk, in_=wkt[hd])
                xbh = xin.tile([F, S], bf16, tag="xbh", name=f"xbh_{hd}")
                nc.sync.dma_start(out=xbh[:, 0:HALF], in_=xtb[hd][:, 0:HALF])
                nc.sync.dma_start(out=xbh[:, HALF:S], in_=xtb[hd][:, HALF:S])
                if QK_FP32R:
                    x32 = xin.tile([F, S], f32r, tag="x32",
                                   name=f"x32_{hd}")
                    nc.sync.dma_start(out=x32, in_=xtq[hd])
                wv = xin.tile([F, F + 1], bf16, tag="wv", name=f"wv_{hd}")
                nc.sync.dma_start(out=wv, in_=wvt[hd])
                bvr = bvb[hd]
                c_bvb = xin.tile([128, GSTRIDE + F + 1], bf16, tag="bvb",
                                 name=f"bvb_{hd}")
                nc.gpsimd.dma_start(
                    out=c_bvb,
                    in_=bass.AP(tensor=bvr.tensor, offset=bvr.offset,
                                ap=[[0, 128]] + list(bvr.ap)))
                st8["bvb"] = c_bvb
                st8["xbh"], st8["wq"], st8["wk"], st8["wv"] = xbh, wq, wk, wv
                st8["xqk"] = x32 if QK_FP32R else xbh
                st8["qt"] = qkp.tile([F, S], qk_dt, tag="qt",
                                     name=f"qt_{hd}")
                st8["kt"] = qkp.tile([F, S], qk_dt, tag="kt",
                                     name=f"kt_{hd}")

            def qk_chunk(which, c, hd=hd, pool=None, tag="vq", act=False):
                wt, bt = ((st8["wq"], c_bq) if which == "q"
                          else (st8["wk"], c_bk))
                dst = st8["qt" if which == "q" else "kt"]
                ps = (pool or vqp).tile([128, 512], f32, tag=tag,
                                        name=f"qk_{hd}_{which}{c}")
                nc.tensor.matmul(
                    ps[:, 0:512], wt[:, :],
                    st8["xqk"][:, 512 * c:512 * (c + 1)],
                    start=True, stop=True)
                if act:  # startup only: ACT is idle, spare the DVE chain
                    nc.scalar.activation(
                        out=dst[:, 512 * c:512 * (c + 1)], in_=ps[:, 0:512],
                        func=mybir.ActivationFunctionType.Identity,
                        bias=bt[:, hd:hd + 1])
                else:
                    nc.vector.tensor_scalar_add(
                        dst[:, 512 * c:512 * (c + 1)], ps[:, 0:512],
                        bt[:, hd:hd + 1])

            def vpd_tile(j, hd=hd):
                # two s-tiles of V' share one PSUM bank (cols 0 and GSTRIDE)
                # and one SBUF tile + one evacuation copy. The second
                # prefill's start=True clears the whole bank's has_written,
                # but pair A is fully accumulated by then (data persists).
                ps = vqp.tile([128, 512], f32, tag="vq",
                              name=f"vps_{hd}_{j}")
                for half_j in range(2):
                    si = 2 * j + half_j
                    g = GSTRIDE * half_j
                    nc.tensor.matmul(
                        ps[:, g:g + F + 1],
                        st8["xbh"][:, 128 * si:128 * (si + 1)],
                        st8["wv"][:, :],
                        start=True, stop=True, skip_group_check=True)
                vt = vpp.tile([128, GSTRIDE + F + 1], bf16, tag="vp",
                              name=f"vp_{hd}_{j}")
                # evacuation copy with the [bv|1] bias folded in
                nc.vector.scalar_tensor_tensor(
                    out=vt[:, :], in0=ps[:, 0:GSTRIDE + F + 1], scalar=1.0,
                    in1=st8["bvb"][:, :], op0=mybir.AluOpType.mult,
                    op1=mybir.AluOpType.add)
                st8["vtiles"].append(vt[:, 0:F + 1])
                st8["vtiles"].append(vt[:, GSTRIDE:GSTRIDE + F + 1])

            # ordered so V' pairs arrive just ahead of the AV batches that
            # need them, and QT/KT chunks ahead of the halves that read
            # them; 14 closures <= 24 k-iterations, so nothing spills to
            # the next head's boundary
            closures = [dmas]
            if hd == 0:
                # startup: spread the first chunks over idle PSUM pools and
                # both ACT+DVE so they run concurrently
                closures.append(lambda: qk_chunk("q", 0, pool=stp, tag="st"))
                closures.append(lambda: qk_chunk("k", 0, pool=avp, tag="av",
                                                 act=True))
                closures.append(lambda: qk_chunk("q", 1, pool=stp, tag="st"))
                closures.append(lambda: qk_chunk("k", 1, pool=avp, tag="av",
                                                 act=True))
                order = [lambda: None, lambda: None,
                         lambda: vpd_tile(0), lambda: vpd_tile(1),
                         lambda c=2: qk_chunk("q", c),
                         lambda c=3: qk_chunk("q", c),
                         lambda c=2: qk_chunk("k", c),
                         lambda c=3: qk_chunk("k", c),
                         lambda: vpd_tile(2), lambda: vpd_tile(3),
                         lambda: vpd_tile(4), lambda: vpd_tile(5),
                         lambda: vpd_tile(6), lambda: vpd_tile(7)]
            else:
                for c in (0, 1):
                    closures.append(lambda c=c: qk_chunk("q", c))
                    closures.append(lambda c=c: qk_chunk("k", c))
                order = [lambda: vpd_tile(0), lambda: vpd_tile(1),
                         lambda c=2: qk_chunk("q", c),
                         lambda c=2: qk_chunk("k", c),
                         lambda: vpd_tile(2),
                         lambda c=3: qk_chunk("q", c),
                         lambda c=3: qk_chunk("k", c),
                         lambda: vpd_tile(3), lambda: vpd_tile(4),
                         lambda: vpd_tile(5), lambda: vpd_tile(6),
                         lambda: vpd_tile(7)]
            closures.extend(order)
            return st8, closures

        head_state = {}
        head_state[0], prelude = make_prelude(0)
        for _ in range(5):  # dmas + q0/k0/q1/k1, on parallel PSUM slots
            prelude.pop(0)()

        for hd in range(HPC):
            if hd > 0:
                while prelude:  # leftovers from the previous k-loop
                    prelude.pop(0)()
            if hd + 1 < HPC:
                head_state[hd + 1], nxt = make_prelude(hd + 1)
                prelude.extend(nxt)
            qt_t = head_state[hd]["qt"]
            kt_t = head_state[hd]["kt"]
            vtiles = head_state[hd]["vtiles"]

            # --- attention, q in two 1024-wide halves ---
            for half in range(2):
                q0 = half * HALF
                nk = (half + 1) * (HALF // 128)  # k-tiles touching this half
                hstate = {}

                for ki in range(nk):
                    ks = 128 * ki
                    ls = max(0, ks - q0)  # local start col within strip
                    strip = stp.tile([128, 1024], f32, tag="st")
                    bounds = [ls, 512, 1024] if ls < 512 else [ls, 1024]
                    pieces = list(zip(bounds[:-1], bounds[1:]))
                    # first ST piece, then the deferred AV batch (whose
                    # matmul stream hides this piece's drain), then the
                    # second piece (its weight load hides under AV drains)
                    nc.tensor.matmul(
                        strip[:, pieces[0][0]:pieces[0][1]],
                        kt_t[:, ks:ks + 128],
                        qt_t[:, q0 + pieces[0][0]:q0 + pieces[0][1]],
                        start=True, stop=True)
                    if prelude:  # hide next head's QKV/V' here
                        prelude.pop(0)()
                    flush_pending(keep=SKEW - 1)
                    for c0, c1 in pieces[1:]:
                        nc.tensor.matmul(
                            strip[:, c0:c1], kt_t[:, ks:ks + 128],
                            qt_t[:, q0 + c0:q0 + c1],
                            start=True, stop=True)
                    ptile = ptp.tile([128, 1024], bf16, tag="pt")
                    nc.scalar.activation(
                        out=ptile[:, ls:1024], in_=strip[:, ls:1024],
                        func=Exp, scale=SCALE)
                    if ks >= q0:  # zero the below-diagonal of the diag block
                        nc.vector.tensor_mul(
                            ptile[:, ls:ls + 128], ptile[:, ls:ls + 128],
                            c_mask[:, :])

                    def av_batch(hd=hd, half=half, ki=ki, ptile=ptile,
                                 hstate=hstate, vtiles=vtiles):
                        if ki == 0:
                            # start=True clears has_written for the WHOLE
                            # bank, so per-group starts would clobber the
                            # other groups packed in the same bank. Clear
                            # each bank once with a dummy matmul into a
                            # spare column; real AV matmuls use start=False
                            # (first write per element overwrites, its bit
                            # being clear).
                            hstate["avts"] = [
                                avp.tile([128, 512], f32, tag="av",
                                         name=f"avacc_{hd}_{half}_{i}")
                                for i in range(3)]
                            for b in range(3):
                                nc.tensor.matmul(
                                    hstate["avts"][b][:, 508:509],
                                    c_one[:, :], c_one[:, 0:1],
                                    start=True, stop=False,
                                    skip_group_check=True)
                        avts = hstate["avts"]
                        for qt in range(max(0, ki - 8 * half), 8):
                            qg = 8 * half + qt
                            g = GSTRIDE * (qt % 3)
                            acc = avts[qt // 3][:, g:g + F + 1]
                            nc.tensor.matmul(
                                acc, ptile[:, 128 * qt:128 * qt + 128],
                                vtiles[ki][:, :],
                                start=False, stop=(ki == qg),
                                skip_group_check=True)
                        # normalize + store once a whole accumulator bank
                        # is finished (avoids PE-write/DVE-read bank overlap)
                        for bank in range(3):
                            last_qt = min(3 * bank + 2, 7)
                            if ki != 8 * half + last_qt:
                                continue
                            ng = last_qt - 3 * bank + 1
                            rc = outp.tile([128, 3], f32, tag="rc")
                            # one strided reciprocal over the bank's sum
                            # columns (at F, F+GSTRIDE, ...)
                            nc.vector.reciprocal(
                                rc[:, 0:ng],
                                avts[bank][:, F:F + 1 + GSTRIDE * (ng - 1):
                                           GSTRIDE])
                            for qt in range(3 * bank, last_qt + 1):
                                qg = 8 * half + qt
                                g = GSTRIDE * (qt % 3)
                                acc = avts[bank][:, g:g + F + 1]
                                ot = outp.tile([128, F], f32, tag="ot")
                                nc.vector.tensor_scalar_mul(
                                    ot[:, :], acc[:, 0:F],
                                    rc[:, qt % 3:qt % 3 + 1])
                                nc.sync.dma_start(
                                    out=out[hd, 128 * qg:128 * (qg + 1), :],
                                    in_=ot[:, :])

                    pending.append(av_batch)
        flush_pending()

    nc.compile()
    return nc


def _prep_inputs(x, Wq, Wk, Wv, bq, bk, bv):
    """Shard + pre-transpose on host. Returns in_maps for 8 cores."""
    bf16 = ml_dtypes.bfloat16
    xf = np.ascontiguousarray(
        x.reshape(B * H, S, F).transpose(0, 2, 1)).astype(np.float32)  # [32,F,S]
    xfb = xf.astype(bf16)
    wqT = np.ascontiguousarray(Wq.transpose(0, 2, 1)).astype(np.float32)  # [H,f,e]
    wkT = np.ascontiguousarray(Wk.transpose(0, 2, 1)).astype(np.float32)
    wvT = np.ascontiguousarray(Wv.transpose(0, 2, 1)).astype(np.float32)
    wvTp = np.zeros((H, F, F + 1), np.float32)
    wvTp[:, :, :F] = wvT
    wvTp = wvTp.astype(bf16)
    bvb_h = np.zeros((H, GSTRIDE + F + 1), np.float32)
    bvb_h[:, 0:F] = bv
    bvb_h[:, F] = 1.0
    bvb_h[:, GSTRIDE:GSTRIDE + F] = bv
    bvb_h[:, GSTRIDE + F] = 1.0
    mask = np.triu(np.ones((F, F), np.float32)).astype(bf16)  # keep r <= c
    ones_row = np.ones((1, F), np.float32).astype(bf16)

    wq_dt = np.float32 if QK_FP32R else bf16
    in_maps = []
    for c in range(NCORES):
        pairs = list(range(HPC * c, HPC * (c + 1)))
        heads = [p % H for p in pairs]
        m = {
            "xtbh": np.ascontiguousarray(xfb[pairs]),
            "wqt": np.ascontiguousarray(wqT[heads]).astype(wq_dt),
            "wkt": np.ascontiguousarray(wkT[heads]).astype(wq_dt),
            "wvt": np.ascontiguousarray(wvTp[heads]),
            "bqt": np.ascontiguousarray(bq[heads].T).astype(np.float32),
            "bkt": np.ascontiguousarray(bk[heads].T).astype(np.float32),
            "bvb": np.ascontiguousarray(bvb_h[heads]).astype(bf16),
            "msk": mask,
            "one": ones_row,
        }
        if QK_FP32R:
            m["xt32"] = np.ascontiguousarray(xf[pairs])
        in_maps.append(m)
    return in_maps


def kernel(x, Wq, Wk, Wv, bq, bk, bv, trace=False):
    x, Wq, Wk, Wv = (np.asarray(a, np.float32) for a in (x, Wq, Wk, Wv))
    bq, bk, bv = (np.asarray(a, np.float32) for a in (bq, bk, bv))

    if "nc" not in _cache:
        _cache["nc"] = _build()
    nc = _cache["nc"]

    in_maps = _prep_inputs(x, Wq, Wk, Wv, bq, bk, bv)
    res = bass_utils.run_bass_kernel_spmd(
        nc, in_maps, core_ids=list(range(NCORES)), trace=trace)

    out = np.empty((B * H, S, F), np.float32)
    for c in range(NCORES):
        out[HPC * c:HPC * (c + 1)] = res.results[c]["out"]
    full = out.reshape(B, H, S, F)
    if trace:
        return full, res
    return full


# revision 55
# speedup vs baseline: 1.0151x; 1.0151x over previous
"""Causal multi-head attention (B=2, H=16, S=2048, F=128) on 8 TRN2 NeuronCores.

Sharding: tensor-parallel over the (batch, head) axis — 32 independent
(b, h) attention problems, 4 per core. No collectives needed.

Per-head on-chip algorithm (all layouts chosen so no on-chip transposes
are ever required):
  - host pre-transposes x to xT [F, S] per head, and W to WT [f, e].
  - QT = WqT.T @ xT   (PSUM) + bias -> SBUF   [e=128, s=2048]
  - KT likewise.
  - V' = xT_tile.T @ [WvT | 0] + [bv | 1]     [s=128, e'=129] per s-tile
    (extra ones column makes the AV matmul also produce softmax denominators)
  - For each k-tile (128 keys), compute ST = K_tile . Q  ->  [k=128, q<=1024]
    strips in PSUM, exp on ACT -> PT (bf16) in SBUF, lower-triangle zero-mask
    on the diagonal block, then AV: out_acc[q,129] += PT_tile.T @ V'_tile,
    accumulated in PSUM over k-tiles. Column 128 of the accumulator is
    sum(exp) — normalize with DVE reciprocal + tensor_scalar multiply.
  - Causality: only k-tiles with k <= q are ever computed.
"""

import math

import numpy as np
import ml_dtypes

import concourse.bass as bass
import concourse.tile as tile
import concourse.mybir as mybir
from concourse import bacc, bass_utils

B, H, S, F = 2, 16, 2048, 128
NCORES = 8
HPC = (B * H) // NCORES  # (b,h) pairs per core
SCALE = 1.0 / math.sqrt(F)
HALF = S // 2  # q processed in two 1024-wide halves (PSUM budget)
GSTRIDE = 136  # col stride of packed AV accumulator groups (32B aligned)

QK_FP32R = False  # scores/projection matmuls in fp32r (else bf16)

_cache = {}


def _build():
    f32 = mybir.dt.float32
    bf16 = mybir.dt.bfloat16
    f32r = mybir.dt.float32r
    qk_dt = f32r if QK_FP32R else bf16
    Exp = mybir.ActivationFunctionType.Exp

    nc = bacc.Bacc("TRN2")

    if QK_FP32R:
        xtq = nc.dram_tensor("xt32", [HPC, F, S], f32r, kind="ExternalInput")
    xtb = nc.dram_tensor("xtbh", [HPC, F, S], bf16, kind="ExternalInput")
    wqt = nc.dram_tensor("wqt", [HPC, F, F], qk_dt, kind="ExternalInput")
    wkt = nc.dram_tensor("wkt", [HPC, F, F], qk_dt, kind="ExternalInput")
    wvt = nc.dram_tensor("wvt", [HPC, F, F + 1], bf16, kind="ExternalInput")
    bqt = nc.dram_tensor("bqt", [F, HPC], f32, kind="ExternalInput")
    bkt = nc.dram_tensor("bkt", [F, HPC], f32, kind="ExternalInput")
    bvb = nc.dram_tensor("bvb", [HPC, GSTRIDE + F + 1], bf16,
                         kind="ExternalInput")
    msk = nc.dram_tensor("msk", [F, F], bf16, kind="ExternalInput")
    one = nc.dram_tensor("one", [1, F], bf16, kind="ExternalInput")
    out = nc.dram_tensor("out", [HPC, S, F], f32, kind="ExternalOutput")
    if not QK_FP32R:
        xtq = xtb

    with tile.TileContext(nc) as tc, \
            tc.tile_pool(name="consts", bufs=1) as consts, \
            tc.tile_pool(name="xin", bufs=2) as xin, \
            tc.tile_pool(name="qk", bufs=2) as qkp, \
            tc.tile_pool(name="vp", bufs=2 * (S // F)) as vpp, \
            tc.tile_pool(name="pt", bufs=7) as ptp, \
            tc.tile_pool(name="outs", bufs=6) as outp, \
            tc.tile_pool(name="st", bufs=2, space="PSUM") as stp, \
            tc.tile_pool(name="av", bufs=3, space="PSUM") as avp, \
            tc.tile_pool(name="vq", bufs=1, space="PSUM") as vqp:

        c_bq = consts.tile([F, HPC], f32, tag="bq")
        nc.gpsimd.dma_start(out=c_bq, in_=bqt[:, :])
        c_bk = consts.tile([F, HPC], f32, tag="bk")
        nc.gpsimd.dma_start(out=c_bk, in_=bkt[:, :])
        c_mask = consts.tile([F, F], bf16, tag="msk")
        nc.gpsimd.dma_start(out=c_mask, in_=msk[:, :])
        c_one = consts.tile([1, F], bf16, tag="one")
        nc.gpsimd.dma_start(out=c_one, in_=one[:, :])

        # deferred AV-batch emission, two strips deep: by the time an AV
        # batch is emitted, the exp it reads finished ~2 iterations ago,
        # so the PE never stalls waiting on ACT
        SKEW = 5
        pending = []

        def flush_pending(keep=0):
            while len(pending) > keep:
                pending.pop(0)()

        def make_prelude(hd):
            """Emission closures for head hd's input DMAs, QT/KT and V'
            projections. Popped one-per-ki during head hd-1's k-loop so
            this work hides under the previous head's softmax."""
            st8 = {"vtiles": []}

            def dmas(hd=hd):
                # halves so the first QK chunk can start after half a load
                wq = xin.tile([F, F], qk_dt, tag="wq", name=f"wq_{hd}")
                nc.sync.dma_start(out=wq, in_=wqt[hd])
                wk = xin.tile([F, F], qk_dt, tag="wk", name=f"wk_{hd}")
                nc.sync.dma_start(out=wk, in_=wkt[hd])
                xbh = xin.tile([F, S], bf16, tag="xbh", name=f"xbh_{hd}")
                nc.sync.dma_start(out=xbh[:, 0:HALF], in_=xtb[hd][:, 0:HALF])
                nc.sync.dma_start(out=xbh[:, HALF:S], in_=xtb[hd][:, HALF:S])
                if QK_FP32R:
                    x32 = xin.tile([F, S], f32r, tag="x32",
                                   name=f"x32_{hd}")
                    nc.sync.dma_start(out=x32, in_=xtq[hd])
                wv = xin.tile([F, F + 1], bf16, tag="wv", name=f"wv_{hd}")
                nc.sync.dma_start(out=wv, in_=wvt[hd])
                bvr = bvb[hd]
                c_bvb = xin.tile([128, GSTRIDE + F + 1], bf16, tag="bvb",
                                 name=f"bvb_{hd}")
                nc.gpsimd.dma_start(
                    out=c_bvb,
                    in_=bass.AP(tensor=bvr.tensor, offset=bvr.offset,
                                ap=[[0, 128]] + list(bvr.ap)))
                st8["bvb"] = c_bvb
                st8["xbh"], st8["wq"], st8["wk"], st8["wv"] = xbh, wq, wk, wv
                st8["xqk"] = x32 if QK_FP32R else xbh
                st8["qt"] = qkp.tile([F, S], qk_dt, tag="qt",
                                     name=f"qt_{hd}")
                st8["kt"] = qkp.tile([F, S], qk_dt, tag="kt",
                                     name=f"kt_{hd}")

            def qk_chunk(which, c, hd=hd, pool=None, tag="vq", act=False):
                wt, bt = ((st8["wq"], c_bq) if which == "q"
                          else (st8["wk"], c_bk))
                dst = st8["qt" if which == "q" else "kt"]
                ps = (pool or vqp).tile([128, 512], f32, tag=tag,
                                        name=f"qk_{hd}_{which}{c}")
                nc.tensor.matmul(
                    ps[:, 0:512], wt[:, :],
                    st8["xqk"][:, 512 * c:512 * (c + 1)],
                    start=True, stop=True)
                if act:  # startup only: ACT is idle, spare the DVE chain
                    nc.scalar.activation(
                        out=dst[:, 512 * c:512 * (c + 1)], in_=ps[:, 0:512],
                        func=mybir.ActivationFunctionType.Identity,
                        bias=bt[:, hd:hd + 1])
                else:
                    nc.vector.tensor_scalar_add(
                        dst[:, 512 * c:512 * (c + 1)], ps[:, 0:512],
                        bt[:, hd:hd + 1])

            def vpd_tile(j, hd=hd):
                # two s-tiles of V' share one PSUM bank (cols 0 and GSTRIDE)
                # and one SBUF tile + one evacuation copy. The second
                # prefill's start=True clears the whole bank's has_written,
                # but pair A is fully accumulated by then (data persists).
                ps = vqp.tile([128, 512], f32, tag="vq",
                              name=f"vps_{hd}_{j}")
                for half_j in range(2):
                    si = 2 * j + half_j
                    g = GSTRIDE * half_j
                    nc.tensor.matmul(
                        ps[:, g:g + F + 1],
                        st8["xbh"][:, 128 * si:128 * (si + 1)],
                        st8["wv"][:, :],
                        start=True, stop=True, skip_group_check=True)
                vt = vpp.tile([128, GSTRIDE + F + 1], bf16, tag="vp",
                              name=f"vp_{hd}_{j}")
                # evacuation copy with the [bv|1] bias folded in
                nc.vector.scalar_tensor_tensor(
                    out=vt[:, :], in0=ps[:, 0:GSTRIDE + F + 1], scalar=1.0,
                    in1=st8["bvb"][:, :], op0=mybir.AluOpType.mult,
                    op1=mybir.AluOpType.add)
                st8["vtiles"].append(vt[:, 0:F + 1])
                st8["vtiles"].append(vt[:, GSTRIDE:GSTRIDE + F + 1])

            # ordered so V' pairs arrive just ahead of the AV batches that
            # need them, and QT/KT chunks ahead of the halves that read
            # them; 14 closures <= 24 k-iterations, so nothing spills to
            # the next head's boundary
            closures = [dmas]
            if hd == 0:
                # startup: spread the first chunks over idle PSUM pools and
                # both ACT+DVE so they run concurrently
                closures.append(lambda: qk_chunk("q", 0, pool=stp, tag="st"))
                closures.append(lambda: qk_chunk("k", 0, pool=avp, tag="av"))
                closures.append(lambda: qk_chunk("q", 1, pool=stp, tag="st"))
                closures.append(lambda: qk_chunk("k", 1, pool=avp, tag="av"))
                order = [lambda: None, lambda: None,
                         lambda: vpd_tile(0), lambda: vpd_tile(1),
                         lambda c=2: qk_chunk("q", c),
                         lambda c=3: qk_chunk("q", c),
                         lambda c=2: qk_chunk("k", c),
                         lambda c=3: qk_chunk("k", c),
                         lambda: vpd_tile(2), lambda: vpd_tile(3),
                         lambda: vpd_tile(4), lambda: vpd_tile(5),
                         lambda: vpd_tile(6), lambda: vpd_tile(7)]
            else:
                for c in (0, 1):
                    closures.append(lambda c=c: qk_chunk("q", c))
                    closures.append(lambda c=c: qk_chunk("k", c))
                order = [lambda: vpd_tile(0), lambda: vpd_tile(1),
                         lambda c=2: qk_chunk("q", c),
                         lambda c=2: qk_chunk("k", c),
                         lambda: vpd_tile(2),
                         lambda c=3: qk_chunk("q", c),
                         lambda c=3: qk_chunk("k", c),
                         lambda: vpd_tile(3), lambda: vpd_tile(4),
                         lambda: vpd_tile(5), lambda: vpd_tile(6),
                         lambda: vpd_tile(7)]
            closures.extend(order)
            return st8, closures

        head_state = {}
        head_state[0], prelude = make_prelude(0)
        for _ in range(5):  # dmas + q0/k0/q1/k1, on parallel PSUM slots
            prelude.pop(0)()

        for hd in range(HPC):
            if hd > 0:
                while prelude:  # leftovers from the previous k-loop
                    prelude.pop(0)()
            if hd + 1 < HPC:
                head_state[hd + 1], nxt = make_prelude(hd + 1)
                prelude.extend(nxt)
            qt_t = head_state[hd]["qt"]
            kt_t = head_state[hd]["kt"]
            vtiles = head_state[hd]["vtiles"]

            # --- attention, q in two 1024-wide halves ---
            for half in range(2):
                q0 = half * HALF
                nk = (half + 1) * (HALF // 128)  # k-tiles touching this half
                hstate = {}

                for ki in range(nk):
                    ks = 128 * ki
                    ls = max(0, ks - q0)  # local start col within strip
                    strip = stp.tile([128, 1024], f32, tag="st")
                    bounds = [ls, 512, 1024] if ls < 512 else [ls, 1024]
                    pieces = list(zip(bounds[:-1], bounds[1:]))
                    # first ST piece, then the deferred AV batch (whose
                    # matmul stream hides this piece's drain), then the
                    # second piece (its weight load hides under AV drains)
                    nc.tensor.matmul(
                        strip[:, pieces[0][0]:pieces[0][1]],
                        kt_t[:, ks:ks + 128],
                        qt_t[:, q0 + pieces[0][0]:q0 + pieces[0][1]],
                        start=True, stop=True)
                    if prelude:  # hide next head's QKV/V' here
                        prelude.pop(0)()
                    flush_pending(keep=SKEW - 1)
                    for c0, c1 in pieces[1:]:
                        nc.tensor.matmul(
                            strip[:, c0:c1], kt_t[:, ks:ks + 128],
                            qt_t[:, q0 + c0:q0 + c1],
                            start=True, stop=True)
                    ptile = ptp.tile([128, 1024], bf16, tag="pt")
                    nc.scalar.activation(
                        out=ptile[:, ls:1024], in_=strip[:, ls:1024],
                        func=Exp, scale=SCALE)
                    if ks >= q0:  # zero the below-diagonal of the diag block
                        nc.vector.tensor_mul(
                            ptile[:, ls:ls + 128], ptile[:, ls:ls + 128],
                            c_mask[:, :])

                    def av_batch(hd=hd, half=half, ki=ki, ptile=ptile,
                                 hstate=hstate, vtiles=vtiles):
                        if ki == 0:
                            # start=True clears has_written for the WHOLE
                            # bank, so per-group starts would clobber the
                            # other groups packed in the same bank. Clear
                            # each bank once with a dummy matmul into a
                            # spare column; real AV matmuls use start=False
                            # (first write per element overwrites, its bit
                            # being clear).
                            hstate["avts"] = [
                                avp.tile([128, 512], f32, tag="av",
                                         name=f"avacc_{hd}_{half}_{i}")
                                for i in range(3)]
                            for b in range(3):
                                nc.tensor.matmul(
                                    hstate["avts"][b][:, 508:509],
                                    c_one[:, :], c_one[:, 0:1],
                                    start=True, stop=False,
                                    skip_group_check=True)
                        avts = hstate["avts"]
                        for qt in range(max(0, ki - 8 * half), 8):
                            qg = 8 * half + qt
                            g = GSTRIDE * (qt % 3)
                            acc = avts[qt // 3][:, g:g + F + 1]
                            nc.tensor.matmul(
                                acc, ptile[:, 128 * qt:128 * qt + 128],
                                vtiles[ki][:, :],
                                start=False, stop=(ki == qg),
                                skip_group_check=True)
                        # normalize + store once a whole accumulator bank
                        # is finished (avoids PE-write/DVE-read bank overlap)
                        for bank in range(3):
                            last_qt = min(3 * bank + 2, 7)
                            if ki != 8 * half + last_qt:
                                continue
                            ng = last_qt - 3 * bank + 1
                            rc = outp.tile([128, 3], f32, tag="rc")
                            # one strided reciprocal over the bank's sum
                            # columns (at F, F+GSTRIDE, ...)
                            nc.vector.reciprocal(
                                rc[:, 0:ng],
                                avts[bank][:, F:F + 1 + GSTRIDE * (ng - 1):
                                           GSTRIDE])
                            for qt in range(3 * bank, last_qt + 1):
                                qg = 8 * half + qt
                                g = GSTRIDE * (qt % 3)
                                acc = avts[bank][:, g:g + F + 1]
                                ot = outp.tile([128, F], f32, tag="ot")
                                nc.vector.tensor_scalar_mul(
                                    ot[:, :], acc[:, 0:F],
                                    rc[:, qt % 3:qt % 3 + 1])
                                nc.sync.dma_start(
                                    out=out[hd, 128 * qg:128 * (qg + 1), :],
                                    in_=ot[:, :])

                    pending.append(av_batch)
        flush_pending()

    nc.compile()
    return nc


def _prep_inputs(x, Wq, Wk, Wv, bq, bk, bv):
    """Shard + pre-transpose on host. Returns in_maps for 8 cores."""
    bf16 = ml_dtypes.bfloat16
    xf = np.ascontiguousarray(
        x.reshape(B * H, S, F).transpose(0, 2, 1)).astype(np.float32)  # [32,F,S]
    xfb = xf.astype(bf16)
    wqT = np.ascontiguousarray(Wq.transpose(0, 2, 1)).astype(np.float32)  # [H,f,e]
    wkT = np.ascontiguousarray(Wk.transpose(0, 2, 1)).astype(np.float32)
    wvT = np.ascontiguousarray(Wv.transpose(0, 2, 1)).astype(np.float32)
    wvTp = np.zeros((H, F, F + 1), np.float32)
    wvTp[:, :, :F] = wvT
    wvTp = wvTp.astype(bf16)
    bvb_h = np.zeros((H, GSTRIDE + F + 1), np.float32)
    bvb_h[:, 0:F] = bv
    bvb_h[:, F] = 1.0
    bvb_h[:, GSTRIDE:GSTRIDE + F] = bv
    bvb_h[:, GSTRIDE + F] = 1.0
    mask = np.triu(np.ones((F, F), np.float32)).astype(bf16)  # keep r <= c
    ones_row = np.ones((1, F), np.float32).astype(bf16)

    wq_dt = np.float32 if QK_FP32R else bf16
    in_maps = []
    for c in range(NCORES):
        pairs = list(range(HPC * c, HPC * (c + 1)))
        heads = [p % H for p in pairs]
        m = {
            "xtbh": np.ascontiguousarray(xfb[pairs]),
            "wqt": np.ascontiguousarray(wqT[heads]).astype(wq_dt),
            "wkt": np.ascontiguousarray(wkT[heads]).astype(wq_dt),
            "wvt": np.ascontiguousarray(wvTp[heads]),
            "bqt": np.ascontiguousarray(bq[heads].T).astype(np.float32),
            "bkt": np.ascontiguousarray(bk[heads].T).astype(np.float32),
            "bvb": np.ascontiguousarray(bvb_h[heads]).astype(bf16),
            "msk": mask,
            "one": ones_row,
        }
        if QK_FP32R:
            m["xt32"] = np.ascontiguousarray(xf[pairs])
        in_maps.append(m)
    return in_maps


def kernel(x, Wq, Wk, Wv, bq, bk, bv, trace=False):
    x, Wq, Wk, Wv = (np.asarray(a, np.float32) for a in (x, Wq, Wk, Wv))
    bq, bk, bv = (np.asarray(a, np.float32) for a in (bq, bk, bv))

    if "nc" not in _cache:
        _cache["nc"] = _build()
    nc = _cache["nc"]

    in_maps = _prep_inputs(x, Wq, Wk, Wv, bq, bk, bv)
    res = bass_utils.run_bass_kernel_spmd(
        nc, in_maps, core_ids=list(range(NCORES)), trace=trace)

    out = np.empty((B * H, S, F), np.float32)
    for c in range(NCORES):
        out[HPC * c:HPC * (c + 1)] = res.results[c]["out"]
    full = out.reshape(B, H, S, F)
    if trace:
        return full, res
    return full
